# revision 1
# baseline (speedup 1.0000x reference)
"""CompressedKVCache kernel for Trainium2 (8 NeuronCores, head-sharded).

Computes, per (b, h) head:
  quantize k/v rows to int4 (per-row min/max affine), scatter into a
  uint8-packed cache at [start_pos : start_pos+L), then dequantize the
  cache prefix [0 : start_pos+L) back to f32.

Sharding: H=32 heads split across 8 cores (4 heads each); everything is
independent per head, no cross-core communication.

The packed cache itself is never returned, so the [start, end) region is
quantize->dequantized entirely on-chip; only the [0, start) prefix is read
from the cache inputs.
"""

import sys

sys.path.insert(0, "/opt/trn_rl_repo")

import numpy as np
from concourse import bass, mybir
from concourse import tile
from concourse.bass_utils import run_bass_kernel_spmd

F32 = mybir.dt.float32
U8 = mybir.dt.uint8
Alu = mybir.AluOpType
Act = mybir.ActivationFunctionType
AX = mybir.AxisListType
MAGIC = float(np.float32(2.0 ** 23))
INV15 = float(np.float32(1.0 / 15.0))

B, H, L, D = 2, 32, 2048, 128
MAX_SEQ = 8192
N_CORES = 8
HC = H // N_CORES  # heads per core


def _split_multiwait(nc):
    """This container's walrus accepts only ONE sync-wait per instruction;
    Tile's tail drain (and occasionally other insts) carry several. Split
    extras into single-wait EventSemaphore insts inserted just before."""
    for fn in nc.m.functions:
        for blk in fn.blocks:
            out = []
            for ins in blk.instructions:
                si = ins.sync_info
                if si is not None and si.on_wait is not None and len(si.on_wait) > 1:
                    waits = list(si.on_wait)
                    for j, w in enumerate(waits[:-1]):
                        out.append(mybir.InstEventSemaphore(
                            name=f"{ins.name}_sw{j}", ins=[], outs=[],
                            engine=ins.engine,
                            sync_info=mybir.SyncInfo(on_wait=[w], on_update=[])))
                    si.on_wait = [waits[-1]]
                    ins.sync_info = si
                out.append(ins)
            blk.instructions = out


def _build(start_pos: int):
    """Trace the per-core Bass kernel for a given start_pos.

    Per core: xk/xv (B,HC,L,D) f32, prefix packed caches (B,HC,S,64) u8 and
    prefix scale/zero rows (B,HC,S) f32 -> ok/ov (B,HC,S+L,D) f32.
    """
    S = start_pos
    E = S + L
    CQ = L // 128            # quant row-chunks per head
    CP = S // 128            # prefix row-chunks per head
    assert L % 128 == 0 and S % 128 == 0 and E <= MAX_SEQ

    nc = bass.Bass(trn_type="TRN2")

    ins_q, ins_p, ins_sc, ins_zp, outs = {}, {}, {}, {}, {}
    for t in ("k", "v"):
        ins_q[t] = nc.dram_tensor(f"x{t}", [B, HC, L, D], F32, kind="ExternalInput")
        if S:
            ins_p[t] = nc.dram_tensor(f"p{t}", [B, HC, S, D // 2], U8, kind="ExternalInput")
            ins_sc[t] = nc.dram_tensor(f"sc{t}", [B, HC, S], F32, kind="ExternalInput")
            ins_zp[t] = nc.dram_tensor(f"zp{t}", [B, HC, S], F32, kind="ExternalInput")
        outs[t] = nc.dram_tensor(f"o{t}", [B, HC, E, D], F32, kind="ExternalOutput")

    U32 = mybir.dt.uint32
    I32 = mybir.dt.int32
    CE = CQ + CP
    with tile.TileContext(nc) as tc:
        with tc.tile_pool(name="big", bufs=3) as big, \
             tc.tile_pool(name="small", bufs=3) as small:
            INF = float(np.float32(3.4e38))
            pair = 0
            for b in range(B):
                for hh in range(HC):
                    act_deq = (pair % 3 != 2)  # offload most pairs' dequant to ACT
                    pair += 1
                    # k/v share stats tiles: k cols [0,CQ), v cols [CQ,2CQ)
                    mn = small.tile([128, 2 * CQ], F32, tag="mn")
                    mx = small.tile([128, 2 * CQ], F32, tag="mx")
                    xs, os_ = {}, {}
                    for kv, t in enumerate(("k", "v")):
                        x_dram = ins_q[t][b, hh, :, :].rearrange("(c p) d -> p c d", p=128)
                        x = big.tile([128, CQ, D], F32, tag=f"x{kv}")
                        nc.sync.dma_start(out=x[:, :, :], in_=x_dram)
                        xs[t] = x
                        os_[t] = big.tile([128, CE, D], F32, tag=f"o{kv}", name=f"o{kv}")
                        # per-chunk min/max reduces (short ops stay under the DVE drain knee)
                        for c in range(CQ):
                            nc.vector.tensor_reduce(out=mx[:, kv * CQ + c:kv * CQ + c + 1],
                                                    in_=x[:, c, :], axis=AX.X, op=Alu.max)
                            nc.vector.tensor_reduce(out=mn[:, kv * CQ + c:kv * CQ + c + 1],
                                                    in_=x[:, c, :], axis=AX.X, op=Alu.min)

                    # one stats chain for both tensors
                    scale = small.tile([128, 2 * CQ], F32, tag="scale")
                    nc.vector.tensor_tensor(out=scale[:, :], in0=mx[:, :], in1=mn[:, :], op=Alu.subtract)
                    nc.vector.tensor_scalar(out=scale[:, :], in0=scale[:, :], scalar1=INV15,
                                            scalar2=1e-8, op0=Alu.mult, op1=Alu.max)
                    rcp = small.tile([128, 2 * CQ], F32, tag="rcp")
                    nc.vector.reciprocal(out=rcp[:, :], in_=scale[:, :])
                    zero = small.tile([128, 2 * CQ], F32, tag="zero")
                    nc.vector.tensor_scalar(out=zero[:, :], in0=mn[:, :], scalar1=-1.0,
                                            scalar2=None, op0=Alu.mult)
                    nc.vector.tensor_tensor(out=zero[:, :], in0=zero[:, :], in1=rcp[:, :], op=Alu.mult)
                    if act_deq:
                        nzs = small.tile([128, 2 * CQ], F32, tag="nzs")
                        nc.vector.tensor_tensor(out=nzs[:, :], in0=zero[:, :], in1=scale[:, :], op=Alu.mult)
                        nc.vector.tensor_scalar(out=nzs[:, :], in0=nzs[:, :], scalar1=-1.0,
                                                scalar2=None, op0=Alu.mult)

                    for kv, t in enumerate(("k", "v")):
                        x, o = xs[t], os_[t]
                        cc0 = kv * CQ
                        # y+round fused: ACT Identity with i32 output (RNE convert)
                        q = big.tile([128, CQ, D], I32, tag=f"q{kv}", bufs=2)
                        for c in range(CQ):
                            nc.scalar.activation(out=q[:, c, :], in_=x[:, c, :], func=Act.Identity,
                                                 bias=zero[:, cc0 + c:cc0 + c + 1],
                                                 scale=rcp[:, cc0 + c:cc0 + c + 1])
                        if act_deq:
                            for c in range(CQ):
                                nc.scalar.activation(out=o[:, CP + c, :], in_=q[:, c, :], func=Act.Identity,
                                                     bias=nzs[:, cc0 + c:cc0 + c + 1],
                                                     scale=scale[:, cc0 + c:cc0 + c + 1])
                        else:
                            for c in range(CQ):
                                nc.vector.tensor_scalar(out=o[:, CP + c, :], in0=q[:, c, :],
                                                        scalar1=zero[:, cc0 + c:cc0 + c + 1],
                                                        scalar2=scale[:, cc0 + c:cc0 + c + 1],
                                                        op0=Alu.subtract, op1=Alu.mult)

                    # ---------------- prefix region [0, S) ----------------
                    if S:
                        sc = small.tile([128, 2 * CP], F32, tag="sc")
                        zp = small.tile([128, 2 * CP], F32, tag="zp")
                        for kv, t in enumerate(("k", "v")):
                            nc.sync.dma_start(out=sc[:, kv * CP:(kv + 1) * CP],
                                              in_=ins_sc[t][b, hh, :].rearrange("(c p) -> p c", p=128))
                            nc.sync.dma_start(out=zp[:, kv * CP:(kv + 1) * CP],
                                              in_=ins_zp[t][b, hh, :].rearrange("(c p) -> p c", p=128))
                        pnzs = small.tile([128, 2 * CP], F32, tag="pnzs")
                        nc.vector.tensor_tensor(out=pnzs[:, :], in0=zp[:, :], in1=sc[:, :], op=Alu.mult)
                        nc.vector.tensor_scalar(out=pnzs[:, :], in0=pnzs[:, :], scalar1=-1.0,
                                                scalar2=None, op0=Alu.mult)

                        for kv, t in enumerate(("k", "v")):
                            o = os_[t]
                            cc0 = kv * CP
                            pk_dram = ins_p[t][b, hh, :, :].rearrange("(c p) d -> p c d", p=128)
                            pk = big.tile([128, CP, D // 2], U8, tag=f"pk{kv}")
                            nc.sync.dma_start(out=pk[:, :, :], in_=pk_dram)
                            # u32-lane nibble unpack: lohi = [lo(64) | hi(64)] per row
                            lohi = big.tile([128, CP, D], U8, tag=f"lohi{kv}")
                            h = CP // 2
                            for g in range(2):  # split ops to stay under the DVE drain knee
                                gs = slice(g * h, (g + 1) * h)
                                pk32 = pk[:, gs, :].bitcast(U32)
                                nc.vector.tensor_scalar(out=lohi[:, gs, 0:D // 2].bitcast(U32), in0=pk32,
                                                        scalar1=0x0F0F0F0F, scalar2=None, op0=Alu.bitwise_and)
                                nc.vector.tensor_scalar(out=lohi[:, gs, D // 2:D].bitcast(U32), in0=pk32,
                                                        scalar1=4, scalar2=0x0F0F0F0F,
                                                        op0=Alu.logical_shift_right, op1=Alu.bitwise_and)
                            # dequant + interleave in one op per chunk (strided out AP)
                            for c in range(CP):
                                src = lohi[:, c, :].rearrange("p (two d) -> p two d", two=2)
                                dst = o[:, c, :].rearrange("p (d two) -> p two d", two=2)
                                nc.vector.tensor_scalar(out=dst, in0=src,
                                                        scalar1=sc[:, cc0 + c:cc0 + c + 1],
                                                        scalar2=pnzs[:, cc0 + c:cc0 + c + 1],
                                                        op0=Alu.mult, op1=Alu.add)

                    for t in ("k", "v"):
                        o_dram = outs[t][b, hh, 0:E, :].rearrange("(c p) d -> p c d", p=128)
                        nc.sync.dma_start(out=o_dram, in_=os_[t][:, :, :])

    _split_multiwait(nc)
    return nc


_CACHE = {}


def _get_nc(start_pos: int):
    if start_pos not in _CACHE:
        _CACHE[start_pos] = _build(start_pos)
    return _CACHE[start_pos]


def _install_ntff_hook_shim():
    """The agent image's antenv lacks axon_hooks; recreate it so
    run_bass_kernel_spmd(trace=True) can drive NTFF profiling."""
    import types
    if "antenv.axon_hooks" in sys.modules:
        return
    mod = types.ModuleType("antenv.axon_hooks")
    state = {"hook": None}
    try:
        from trn_agent_boot.trn_boot import _ntff_profile_via_ctypes
        state["hook"] = _ntff_profile_via_ctypes("/opt/axon/libaxon_pjrt.so")
    except Exception:
        pass
    mod.get_axon_ntff_profile_hook = lambda: state["hook"]
    mod.set_axon_ntff_profile_hook = lambda h: state.__setitem__("hook", h)
    sys.modules["antenv.axon_hooks"] = mod


def _kernel_np(k, v, k_cache, v_cache, k_scale, k_zero, v_scale, v_zero, start_pos):
    """Pure-numpy fallback for shapes the bass path doesn't handle."""
    def qp(x):
        mn = x.min(-1, keepdims=True)
        mx = x.max(-1, keepdims=True)
        scale = np.maximum((mx - mn) / np.float32(15.0), np.float32(1e-8))
        zero = -mn / scale
        q = np.clip(np.round(x / scale + zero), 0, 15).astype(np.uint8)
        return (q[..., 0::2] | (q[..., 1::2] << 4)), scale[..., 0], zero[..., 0]

    def dq(p, s, z):
        lo = (p & 15).astype(np.float32)
        hi = ((p >> 4) & 15).astype(np.float32)
        q = np.stack([lo, hi], -1).reshape(p.shape[:-1] + (p.shape[-1] * 2,))
        return (q - z[..., None]) * s[..., None]

    S = int(start_pos)
    E = S + k.shape[2]
    outs = []
    for x, cache, sc, zp in ((k, k_cache, k_scale, k_zero), (v, v_cache, v_scale, v_zero)):
        pp, ps, pz = qp(x)
        cache = cache.copy(); sc = sc.copy(); zp = zp.copy()
        cache[:, :, S:E] = pp
        sc[:, :, S:E] = ps
        zp[:, :, S:E] = pz
        outs.append(dq(cache[:, :, :E], sc[:, :, :E], zp[:, :, :E]))
    return tuple(outs)


def kernel(k, v, k_cache, v_cache, k_scale, k_zero, v_scale, v_zero, start_pos,
           _trace=False):
    k = np.asarray(k, np.float32)
    v = np.asarray(v, np.float32)
    k_cache = np.asarray(k_cache, np.uint8)
    v_cache = np.asarray(v_cache, np.uint8)
    k_scale = np.asarray(k_scale, np.float32)
    k_zero = np.asarray(k_zero, np.float32)
    v_scale = np.asarray(v_scale, np.float32)
    v_zero = np.asarray(v_zero, np.float32)
    S = int(start_pos)

    if (k.shape != (B, H, L, D) or S % 128 or S + L > MAX_SEQ):
        return _kernel_np(k, v, k_cache, v_cache, k_scale, k_zero, v_scale, v_zero, S)

    nc = _get_nc(S)
    E = S + L

    in_maps = []
    for m in range(N_CORES):
        hs = slice(m * HC, (m + 1) * HC)
        im = {
            "xk": np.ascontiguousarray(k[:, hs]),
            "xv": np.ascontiguousarray(v[:, hs]),
        }
        if S:
            im["pk"] = np.ascontiguousarray(k_cache[:, hs, :S, :])
            im["pv"] = np.ascontiguousarray(v_cache[:, hs, :S, :])
            im["sck"] = np.ascontiguousarray(k_scale[:, hs, :S])
            im["zpk"] = np.ascontiguousarray(k_zero[:, hs, :S])
            im["scv"] = np.ascontiguousarray(v_scale[:, hs, :S])
            im["zpv"] = np.ascontiguousarray(v_zero[:, hs, :S])
        in_maps.append(im)

    if _trace:
        _install_ntff_hook_shim()
    res = run_bass_kernel_spmd(nc, in_maps, list(range(N_CORES)), trace=_trace)

    k_dec = np.empty((B, H, E, D), np.float32)
    v_dec = np.empty((B, H, E, D), np.float32)
    for m in range(N_CORES):
        hs = slice(m * HC, (m + 1) * HC)
        k_dec[:, hs] = res.results[m]["ok"]
        v_dec[:, hs] = res.results[m]["ov"]
    if _trace:
        return (k_dec, v_dec), res
    return k_dec, v_dec



# revision 2
# speedup vs baseline: 1.2314x; 1.2314x over previous
"""CompressedKVCache kernel for Trainium2 (8 NeuronCores, head-sharded).

Computes, per (b, h) head:
  quantize k/v rows to int4 (per-row min/max affine), scatter into a
  uint8-packed cache at [start_pos : start_pos+L), then dequantize the
  cache prefix [0 : start_pos+L) back to f32.

Sharding: H=32 heads split across 8 cores (4 heads each); everything is
independent per head, no cross-core communication.

The packed cache itself is never returned, so the [start, end) region is
quantize->dequantized entirely on-chip; only the [0, start) prefix is read
from the cache inputs.

v2 layout: all DRAM<->SBUF transfers use a "(p c)" row blocking (partition
p owns 16 *consecutive* rows as column chunks) so every DMA descriptor is
a large contiguous run (1-8 KiB) instead of the 4-512 B descriptors the
interleaved "(c p)" layout generates.  Min/max reduces are batched into
multi-chunk ops; the prefix dequant runs on the otherwise-idle GpSimd
engine; quant stays on ACT (free RNE round via i32 output); the quant
region dequant runs on DVE.
"""

import sys

sys.path.insert(0, "/opt/trn_rl_repo")

import numpy as np
from concourse import bass, mybir
from concourse import tile
from concourse.bass_utils import run_bass_kernel_spmd

F32 = mybir.dt.float32
U8 = mybir.dt.uint8
U32 = mybir.dt.uint32
I32 = mybir.dt.int32
Alu = mybir.AluOpType
Act = mybir.ActivationFunctionType
AX = mybir.AxisListType
INV15 = float(np.float32(1.0 / 15.0))

B, H, L, D = 2, 32, 2048, 128
MAX_SEQ = 8192
N_CORES = 8
HC = H // N_CORES  # heads per core

# --- tuning knobs -----------------------------------------------------------
RG = 8            # chunks per min/max reduce op (divisor of 32)
DQR_ENGINE = "vector"    # engine for quant-region dequant
PRE_ENGINE = "gpsimd"    # engine for prefix dequant+interleave
UNPACK_ENGINE = "vector" # engine for nibble unpack


def _split_multiwait(nc):
    """This container's walrus accepts only ONE sync-wait per instruction;
    Tile's tail drain (and occasionally other insts) carry several. Split
    extras into single-wait EventSemaphore insts inserted just before."""
    for fn in nc.m.functions:
        for blk in fn.blocks:
            out = []
            for ins in blk.instructions:
                si = ins.sync_info
                if si is not None and si.on_wait is not None and len(si.on_wait) > 1:
                    waits = list(si.on_wait)
                    for j, w in enumerate(waits[:-1]):
                        out.append(mybir.InstEventSemaphore(
                            name=f"{ins.name}_sw{j}", ins=[], outs=[],
                            engine=ins.engine,
                            sync_info=mybir.SyncInfo(on_wait=[w], on_update=[])))
                    si.on_wait = [waits[-1]]
                    ins.sync_info = si
                out.append(ins)
            blk.instructions = out


def _build(start_pos: int):
    """Trace the per-core Bass kernel for a given start_pos.

    Per core: xk/xv (B,HC,L,D) f32, prefix packed caches (B,HC,S,64) u8 and
    prefix scale/zero rows (B,HC,S) f32 -> ok/ov (B,HC,S+L,D) f32.
    """
    S = start_pos
    E = S + L
    CQ = L // 128            # quant row-chunks per head
    CP = S // 128            # prefix row-chunks per head
    assert L % 128 == 0 and S % 128 == 0 and E <= MAX_SEQ

    nc = bass.Bass(trn_type="TRN2")

    ins_q, ins_p, ins_sc, ins_zp, outs = {}, {}, {}, {}, {}
    for t in ("k", "v"):
        ins_q[t] = nc.dram_tensor(f"x{t}", [B, HC, L, D], F32, kind="ExternalInput")
        if S:
            ins_p[t] = nc.dram_tensor(f"p{t}", [B, HC, S, D // 2], U8, kind="ExternalInput")
            ins_sc[t] = nc.dram_tensor(f"sc{t}", [B, HC, S], F32, kind="ExternalInput")
            ins_zp[t] = nc.dram_tensor(f"zp{t}", [B, HC, S], F32, kind="ExternalInput")
        outs[t] = nc.dram_tensor(f"o{t}", [B, HC, E, D], F32, kind="ExternalOutput")

    with tile.TileContext(nc) as tc:
        with tc.tile_pool(name="big", bufs=3) as big, \
             tc.tile_pool(name="small", bufs=3) as small:
            eng = {"vector": nc.vector, "gpsimd": nc.gpsimd}
            for b in range(B):
                for hh in range(HC):
                    # ---- load quant inputs: partition p <- rows p*16..p*16+15
                    xkv = big.tile([128, 2 * CQ, D], F32, tag="xkv")
                    for kv, t in enumerate(("k", "v")):
                        x_dram = ins_q[t][b, hh, :, :].rearrange(
                            "(p c) d -> p c d", p=128)
                        nc.sync.dma_start(
                            out=xkv[:, kv * CQ:(kv + 1) * CQ, :], in_=x_dram)

                    # ---- batched min/max reduces (DVE)
                    mn = small.tile([128, 2 * CQ], F32, tag="mn")
                    mx = small.tile([128, 2 * CQ], F32, tag="mx")
                    for g in range(0, 2 * CQ, RG):
                        nc.vector.tensor_reduce(
                            out=mx[:, g:g + RG], in_=xkv[:, g:g + RG, :],
                            axis=AX.X, op=Alu.max)
                        nc.vector.tensor_reduce(
                            out=mn[:, g:g + RG], in_=xkv[:, g:g + RG, :],
                            axis=AX.X, op=Alu.min)

                    # ---- stats chain (DVE, small ops)
                    scale = small.tile([128, 2 * CQ], F32, tag="scale")
                    nc.vector.tensor_tensor(out=scale[:, :], in0=mx[:, :],
                                            in1=mn[:, :], op=Alu.subtract)
                    nc.vector.tensor_scalar(out=scale[:, :], in0=scale[:, :],
                                            scalar1=INV15, scalar2=1e-8,
                                            op0=Alu.mult, op1=Alu.max)
                    rcp = small.tile([128, 2 * CQ], F32, tag="rcp")
                    nc.vector.reciprocal(out=rcp[:, :], in_=scale[:, :])
                    zero = small.tile([128, 2 * CQ], F32, tag="zero")
                    nc.vector.tensor_scalar(out=zero[:, :], in0=mn[:, :],
                                            scalar1=-1.0, scalar2=None,
                                            op0=Alu.mult)
                    nc.vector.tensor_tensor(out=zero[:, :], in0=zero[:, :],
                                            in1=rcp[:, :], op=Alu.mult)

                    # ---- output tiles: [r=0] prefix rows, [r=1] quant rows
                    os_ = {}
                    for kv, t in enumerate(("k", "v")):
                        os_[t] = big.tile([128, 2, CP, D], F32,
                                          tag=f"o{kv}", name=f"o{kv}")

                    # ---- quant: ACT Identity with i32 out (RNE round),
                    #      in-place into xkv (reduces already consumed x)
                    qdst = xkv.bitcast(I32)
                    for kv, t in enumerate(("k", "v")):
                        for c in range(CQ):
                            cc = kv * CQ + c
                            nc.scalar.activation(
                                out=qdst[:, cc, :], in_=xkv[:, cc, :],
                                func=Act.Identity,
                                bias=zero[:, cc:cc + 1],
                                scale=rcp[:, cc:cc + 1])

                    # ---- quant-region dequant: (q - zero) * scale
                    dq_eng = eng[DQR_ENGINE]
                    for kv, t in enumerate(("k", "v")):
                        o = os_[t]
                        for c in range(CQ):
                            cc = kv * CQ + c
                            dq_eng.tensor_scalar(
                                out=o[:, 1, c, :], in0=qdst[:, cc, :],
                                scalar1=zero[:, cc:cc + 1],
                                scalar2=scale[:, cc:cc + 1],
                                op0=Alu.subtract, op1=Alu.mult)

                    # ---------------- prefix region [0, S) ----------------
                    if S:
                        # scales/zeros: partition p <- rows p*16..p*16+15
                        sc = small.tile([128, 2 * CP], F32, tag="sc")
                        zp = small.tile([128, 2 * CP], F32, tag="zp")
                        for kv, t in enumerate(("k", "v")):
                            nc.sync.dma_start(
                                out=sc[:, kv * CP:(kv + 1) * CP],
                                in_=ins_sc[t][b, hh, :].rearrange(
                                    "(p c) -> p c", p=128))
                            nc.sync.dma_start(
                                out=zp[:, kv * CP:(kv + 1) * CP],
                                in_=ins_zp[t][b, hh, :].rearrange(
                                    "(p c) -> p c", p=128))
                        pnz = small.tile([128, 2 * CP], F32, tag="pnz")
                        nc.vector.tensor_tensor(out=pnz[:, :], in0=zp[:, :],
                                                in1=sc[:, :], op=Alu.mult)
                        nc.vector.tensor_scalar(out=pnz[:, :], in0=pnz[:, :],
                                                scalar1=-1.0, scalar2=None,
                                                op0=Alu.mult)

                        pk2 = big.tile([128, 2, CP, D // 2], U8, tag="pk")
                        lohi = big.tile([128, 2, CP, D], U8, tag="lohi")
                        up_eng = eng[UNPACK_ENGINE]
                        pre_eng = eng[PRE_ENGINE]
                        for kv, t in enumerate(("k", "v")):
                            nc.sync.dma_start(
                                out=pk2[:, kv, :, :],
                                in_=ins_p[t][b, hh, :, :].rearrange(
                                    "(p c) d -> p c d", p=128))
                            # u32-lane nibble unpack: lo -> cols 0:64,
                            # hi -> cols 64:128 per row
                            pk32 = pk2[:, kv, :, :].bitcast(U32)
                            up_eng.tensor_scalar(
                                out=lohi[:, kv, :, 0:D // 2].bitcast(U32),
                                in0=pk32, scalar1=0x0F0F0F0F, scalar2=None,
                                op0=Alu.bitwise_and)
                            up_eng.tensor_scalar(
                                out=lohi[:, kv, :, D // 2:D].bitcast(U32),
                                in0=pk32, scalar1=4, scalar2=0x0F0F0F0F,
                                op0=Alu.logical_shift_right,
                                op1=Alu.bitwise_and)
                            # dequant + interleave per chunk (strided dst)
                            o = os_[t]
                            for c in range(CP):
                                cc = kv * CP + c
                                src = lohi[:, kv, c, :].rearrange(
                                    "p (two d) -> p two d", two=2)
                                dst = o[:, 0, c, :].rearrange(
                                    "p (d two) -> p two d", two=2)
                                pre_eng.tensor_scalar(
                                    out=dst, in0=src,
                                    scalar1=sc[:, cc:cc + 1],
                                    scalar2=pnz[:, cc:cc + 1],
                                    op0=Alu.mult, op1=Alu.add)

                    # ---- stores: one DMA per tensor, 2 contiguous 8 KiB
                    #      runs per partition
                    for t in ("k", "v"):
                        o_dram = outs[t][b, hh, 0:E, :].rearrange(
                            "(r p c) d -> p r c d", r=2, p=128)
                        nc.sync.dma_start(out=o_dram, in_=os_[t][:, :, :, :])

    _split_multiwait(nc)
    return nc


_CACHE = {}


def _get_nc(start_pos: int):
    if start_pos not in _CACHE:
        _CACHE[start_pos] = _build(start_pos)
    return _CACHE[start_pos]


def _install_ntff_hook_shim():
    """The agent image's antenv lacks axon_hooks; recreate it so
    run_bass_kernel_spmd(trace=True) can drive NTFF profiling."""
    import types
    if "antenv.axon_hooks" in sys.modules:
        return
    mod = types.ModuleType("antenv.axon_hooks")
    state = {"hook": None}
    try:
        from trn_agent_boot.trn_boot import _ntff_profile_via_ctypes
        state["hook"] = _ntff_profile_via_ctypes("/opt/axon/libaxon_pjrt.so")
    except Exception:
        pass
    mod.get_axon_ntff_profile_hook = lambda: state["hook"]
    mod.set_axon_ntff_profile_hook = lambda h: state.__setitem__("hook", h)
    sys.modules["antenv.axon_hooks"] = mod


def _kernel_np(k, v, k_cache, v_cache, k_scale, k_zero, v_scale, v_zero, start_pos):
    """Pure-numpy fallback for shapes the bass path doesn't handle."""
    def qp(x):
        mn = x.min(-1, keepdims=True)
        mx = x.max(-1, keepdims=True)
        scale = np.maximum((mx - mn) / np.float32(15.0), np.float32(1e-8))
        zero = -mn / scale
        q = np.clip(np.round(x / scale + zero), 0, 15).astype(np.uint8)
        return (q[..., 0::2] | (q[..., 1::2] << 4)), scale[..., 0], zero[..., 0]

    def dq(p, s, z):
        lo = (p & 15).astype(np.float32)
        hi = ((p >> 4) & 15).astype(np.float32)
        q = np.stack([lo, hi], -1).reshape(p.shape[:-1] + (p.shape[-1] * 2,))
        return (q - z[..., None]) * s[..., None]

    S = int(start_pos)
    E = S + k.shape[2]
    outs = []
    for x, cache, sc, zp in ((k, k_cache, k_scale, k_zero), (v, v_cache, v_scale, v_zero)):
        pp, ps, pz = qp(x)
        cache = cache.copy(); sc = sc.copy(); zp = zp.copy()
        cache[:, :, S:E] = pp
        sc[:, :, S:E] = ps
        zp[:, :, S:E] = pz
        outs.append(dq(cache[:, :, :E], sc[:, :, :E], zp[:, :, :E]))
    return tuple(outs)


def kernel(k, v, k_cache, v_cache, k_scale, k_zero, v_scale, v_zero, start_pos,
           _trace=False):
    k = np.asarray(k, np.float32)
    v = np.asarray(v, np.float32)
    k_cache = np.asarray(k_cache, np.uint8)
    v_cache = np.asarray(v_cache, np.uint8)
    k_scale = np.asarray(k_scale, np.float32)
    k_zero = np.asarray(k_zero, np.float32)
    v_scale = np.asarray(v_scale, np.float32)
    v_zero = np.asarray(v_zero, np.float32)
    S = int(start_pos)

    if (k.shape != (B, H, L, D) or S % 128 or S + L > MAX_SEQ):
        return _kernel_np(k, v, k_cache, v_cache, k_scale, k_zero, v_scale, v_zero, S)

    nc = _get_nc(S)
    E = S + L

    in_maps = []
    for m in range(N_CORES):
        hs = slice(m * HC, (m + 1) * HC)
        im = {
            "xk": np.ascontiguousarray(k[:, hs]),
            "xv": np.ascontiguousarray(v[:, hs]),
        }
        if S:
            im["pk"] = np.ascontiguousarray(k_cache[:, hs, :S, :])
            im["pv"] = np.ascontiguousarray(v_cache[:, hs, :S, :])
            im["sck"] = np.ascontiguousarray(k_scale[:, hs, :S])
            im["zpk"] = np.ascontiguousarray(k_zero[:, hs, :S])
            im["scv"] = np.ascontiguousarray(v_scale[:, hs, :S])
            im["zpv"] = np.ascontiguousarray(v_zero[:, hs, :S])
        in_maps.append(im)

    if _trace:
        _install_ntff_hook_shim()
    res = run_bass_kernel_spmd(nc, in_maps, list(range(N_CORES)), trace=_trace)

    k_dec = np.empty((B, H, E, D), np.float32)
    v_dec = np.empty((B, H, E, D), np.float32)
    for m in range(N_CORES):
        hs = slice(m * HC, (m + 1) * HC)
        k_dec[:, hs] = res.results[m]["ok"]
        v_dec[:, hs] = res.results[m]["ov"]
    if _trace:
        return (k_dec, v_dec), res
    return k_dec, v_dec


# revision 5
# speedup vs baseline: 1.2375x; 1.0050x over previous
"""CompressedKVCache kernel for Trainium2 (8 NeuronCores, head-sharded).

Computes, per (b, h) head:
  quantize k/v rows to int4 (per-row min/max affine), scatter into a
  uint8-packed cache at [start_pos : start_pos+L), then dequantize the
  cache prefix [0 : start_pos+L) back to f32.

Sharding: H=32 heads split across 8 cores (4 heads each); everything is
independent per head, no cross-core communication.

The packed cache itself is never returned, so the [start, end) region is
quantize->dequantized entirely on-chip; only the [0, start) prefix is read
from the cache inputs.

Layout: all DRAM<->SBUF transfers use a "(p c)" row blocking (partition p
owns 16 *consecutive* rows as column chunks) so every DMA descriptor is a
large contiguous run (1-8 KiB).  Engine balance: min/max reduces + most of
the quant-region dequant on DVE, quant round (f32->u8 RNE) + the rest of
the dequant on ACT, nibble unpack + prefix dequant on GpSimd.
"""

import sys

sys.path.insert(0, "/opt/trn_rl_repo")

import numpy as np
from concourse import bass, mybir
from concourse import tile
from concourse.bass_utils import run_bass_kernel_spmd

F32 = mybir.dt.float32
U8 = mybir.dt.uint8
U32 = mybir.dt.uint32
I32 = mybir.dt.int32
Alu = mybir.AluOpType
Act = mybir.ActivationFunctionType
AX = mybir.AxisListType
INV15 = float(np.float32(1.0 / 15.0))

B, H, L, D = 2, 32, 2048, 128
MAX_SEQ = 8192
N_CORES = 8
HC = H // N_CORES  # heads per core

# --- tuning knobs -----------------------------------------------------------
RG = 8           # chunks per min/max reduce op (divisor of 16)
DQR_ACT = 4      # per (pair, tensor): dequant chunks on ACT; rest on DVE
PRE_DVE = 0      # per (pair, tensor): prefix-dequant chunks on DVE; rest GpSimd
Q_DTYPE = U8     # quantized value dtype (ACT output convert does the round)


def _split_multiwait(nc):
    """This container's walrus accepts only ONE sync-wait per instruction;
    Tile's tail drain (and occasionally other insts) carry several. Split
    extras into single-wait EventSemaphore insts inserted just before."""
    for fn in nc.m.functions:
        for blk in fn.blocks:
            out = []
            for ins in blk.instructions:
                si = ins.sync_info
                if si is not None and si.on_wait is not None and len(si.on_wait) > 1:
                    waits = list(si.on_wait)
                    for j, w in enumerate(waits[:-1]):
                        out.append(mybir.InstEventSemaphore(
                            name=f"{ins.name}_sw{j}", ins=[], outs=[],
                            engine=ins.engine,
                            sync_info=mybir.SyncInfo(on_wait=[w], on_update=[])))
                    si.on_wait = [waits[-1]]
                    ins.sync_info = si
                out.append(ins)
            blk.instructions = out


def _build(start_pos: int):
    """Trace the per-core Bass kernel for a given start_pos.

    Per core: xk/xv (B,HC,L,D) f32, prefix packed caches (B,HC,S,64) u8 and
    prefix scale/zero rows (B,HC,S) f32 -> ok/ov (B,HC,S+L,D) f32.
    """
    S = start_pos
    E = S + L
    CQ = L // 128            # quant row-chunks per head
    CP = S // 128            # prefix row-chunks per head
    assert L % 128 == 0 and S % 128 == 0 and E <= MAX_SEQ

    nc = bass.Bass(trn_type="TRN2")

    ins_q, ins_p, ins_sc, ins_zp, outs = {}, {}, {}, {}, {}
    for t in ("k", "v"):
        ins_q[t] = nc.dram_tensor(f"x{t}", [B, HC, L, D], F32, kind="ExternalInput")
        if S:
            ins_p[t] = nc.dram_tensor(f"p{t}", [B, HC, S, D // 2], U8, kind="ExternalInput")
            ins_sc[t] = nc.dram_tensor(f"sc{t}", [B, HC, S], F32, kind="ExternalInput")
            ins_zp[t] = nc.dram_tensor(f"zp{t}", [B, HC, S], F32, kind="ExternalInput")
        outs[t] = nc.dram_tensor(f"o{t}", [B, HC, E, D], F32, kind="ExternalOutput")

    with tile.TileContext(nc) as tc:
        with tc.tile_pool(name="big", bufs=3) as big, \
             tc.tile_pool(name="small", bufs=3) as small:
            for b in range(B):
                for hh in range(HC):
                    # ---- prefix inputs first: feed the GpSimd pipeline
                    if S:
                        sc = small.tile([128, 2 * CP], F32, tag="sc")
                        zp = small.tile([128, 2 * CP], F32, tag="zp")
                        for kv, t in enumerate(("k", "v")):
                            nc.sync.dma_start(
                                out=sc[:, kv * CP:(kv + 1) * CP],
                                in_=ins_sc[t][b, hh, :].rearrange(
                                    "(p c) -> p c", p=128))
                            nc.sync.dma_start(
                                out=zp[:, kv * CP:(kv + 1) * CP],
                                in_=ins_zp[t][b, hh, :].rearrange(
                                    "(p c) -> p c", p=128))
                        pk2 = big.tile([128, 2, CP, D // 2], U8, tag="pk")
                        for kv, t in enumerate(("k", "v")):
                            nc.sync.dma_start(
                                out=pk2[:, kv, :, :],
                                in_=ins_p[t][b, hh, :, :].rearrange(
                                    "(p c) d -> p c d", p=128))

                    # ---- load quant inputs: partition p <- rows p*16..p*16+15
                    xkv = big.tile([128, 2 * CQ, D], F32, tag="xkv")
                    for kv, t in enumerate(("k", "v")):
                        x_dram = ins_q[t][b, hh, :, :].rearrange(
                            "(p c) d -> p c d", p=128)
                        nc.sync.dma_start(
                            out=xkv[:, kv * CQ:(kv + 1) * CQ, :], in_=x_dram)

                    # ---- prefix stats + unpack + dequant (GpSimd path)
                    os_p, os_q = {}, {}
                    for kv, t in enumerate(("k", "v")):
                        os_p[t] = big.tile([128, CP, D], F32, tag=f"op{kv}",
                                           name=f"op{kv}")
                        os_q[t] = big.tile([128, CQ, D], F32, tag=f"oq{kv}",
                                           name=f"oq{kv}")
                    if S:
                        pnz = small.tile([128, 2 * CP], F32, tag="pnz")
                        nc.vector.tensor_tensor(out=pnz[:, :], in0=zp[:, :],
                                                in1=sc[:, :], op=Alu.mult)
                        nc.vector.tensor_scalar(out=pnz[:, :], in0=pnz[:, :],
                                                scalar1=-1.0, scalar2=None,
                                                op0=Alu.mult)
                        lohi = big.tile([128, 2, CP, D], U8, tag="lohi")
                        for kv, t in enumerate(("k", "v")):
                            # u32-lane nibble unpack: lo -> cols 0:64,
                            # hi -> cols 64:128 per row
                            pk32 = pk2[:, kv, :, :].bitcast(U32)
                            nc.vector.tensor_scalar(
                                out=lohi[:, kv, :, 0:D // 2].bitcast(U32),
                                in0=pk32, scalar1=0x0F0F0F0F, scalar2=None,
                                op0=Alu.bitwise_and)
                            nc.vector.tensor_scalar(
                                out=lohi[:, kv, :, D // 2:D].bitcast(U32),
                                in0=pk32, scalar1=4, scalar2=0x0F0F0F0F,
                                op0=Alu.logical_shift_right,
                                op1=Alu.bitwise_and)
                            # dequant + interleave per chunk (strided dst)
                            o = os_p[t]
                            for c in range(CP):
                                cc = kv * CP + c
                                src = lohi[:, kv, c, :].rearrange(
                                    "p (two d) -> p two d", two=2)
                                dst = o[:, c, :].rearrange(
                                    "p (d two) -> p two d", two=2)
                                eng = nc.vector if c < PRE_DVE else nc.gpsimd
                                eng.tensor_scalar(
                                    out=dst, in0=src,
                                    scalar1=sc[:, cc:cc + 1],
                                    scalar2=pnz[:, cc:cc + 1],
                                    op0=Alu.mult, op1=Alu.add)
                            # prefix store fires as soon as this half is done
                            o_dram = outs[t][b, hh, 0:S, :].rearrange(
                                "(p c) d -> p c d", p=128)
                            nc.sync.dma_start(out=o_dram, in_=o[:, :, :])

                    # ---- batched min/max reduces (DVE)
                    mn = small.tile([128, 2 * CQ], F32, tag="mn")
                    mx = small.tile([128, 2 * CQ], F32, tag="mx")
                    for g in range(0, 2 * CQ, RG):
                        nc.vector.tensor_reduce(
                            out=mx[:, g:g + RG], in_=xkv[:, g:g + RG, :],
                            axis=AX.X, op=Alu.max)
                        nc.vector.tensor_reduce(
                            out=mn[:, g:g + RG], in_=xkv[:, g:g + RG, :],
                            axis=AX.X, op=Alu.min)

                    # ---- stats chain (DVE, small ops)
                    scale = small.tile([128, 2 * CQ], F32, tag="scale")
                    nc.vector.tensor_tensor(out=scale[:, :], in0=mx[:, :],
                                            in1=mn[:, :], op=Alu.subtract)
                    nc.vector.tensor_scalar(out=scale[:, :], in0=scale[:, :],
                                            scalar1=INV15, scalar2=1e-8,
                                            op0=Alu.mult, op1=Alu.max)
                    rcp = small.tile([128, 2 * CQ], F32, tag="rcp")
                    nc.vector.reciprocal(out=rcp[:, :], in_=scale[:, :])
                    zero = small.tile([128, 2 * CQ], F32, tag="zero")
                    nc.vector.tensor_scalar(out=zero[:, :], in0=mn[:, :],
                                            scalar1=-1.0, scalar2=None,
                                            op0=Alu.mult)
                    nc.vector.tensor_tensor(out=zero[:, :], in0=zero[:, :],
                                            in1=rcp[:, :], op=Alu.mult)

                    # ---- quant: ACT Identity, u8 out (RNE round in convert)
                    q8 = big.tile([128, 2 * CQ, D], Q_DTYPE, tag="q")
                    for kv, t in enumerate(("k", "v")):
                        for c in range(CQ):
                            cc = kv * CQ + c
                            nc.scalar.activation(
                                out=q8[:, cc, :], in_=xkv[:, cc, :],
                                func=Act.Identity,
                                bias=zero[:, cc:cc + 1],
                                scale=rcp[:, cc:cc + 1])

                    # ---- quant-region dequant, split ACT / DVE
                    for kv, t in enumerate(("k", "v")):
                        o = os_q[t]
                        for c in range(CQ):
                            cc = kv * CQ + c
                            if c < DQR_ACT:
                                # q*scale + mn  (== (q - zero)*scale)
                                nc.scalar.activation(
                                    out=o[:, c, :], in_=q8[:, cc, :],
                                    func=Act.Identity,
                                    bias=mn[:, cc:cc + 1],
                                    scale=scale[:, cc:cc + 1])
                            else:
                                nc.vector.tensor_scalar(
                                    out=o[:, c, :], in0=q8[:, cc, :],
                                    scalar1=zero[:, cc:cc + 1],
                                    scalar2=scale[:, cc:cc + 1],
                                    op0=Alu.subtract, op1=Alu.mult)
                        o_dram = outs[t][b, hh, S:E, :].rearrange(
                            "(p c) d -> p c d", p=128)
                        nc.sync.dma_start(out=o_dram, in_=o[:, :, :])

    _split_multiwait(nc)
    return nc


_CACHE = {}


def _get_nc(start_pos: int):
    if start_pos not in _CACHE:
        _CACHE[start_pos] = _build(start_pos)
    return _CACHE[start_pos]


def _install_ntff_hook_shim():
    """The agent image's antenv lacks axon_hooks; recreate it so
    run_bass_kernel_spmd(trace=True) can drive NTFF profiling."""
    import types
    if "antenv.axon_hooks" in sys.modules:
        return
    mod = types.ModuleType("antenv.axon_hooks")
    state = {"hook": None}
    try:
        from trn_agent_boot.trn_boot import _ntff_profile_via_ctypes
        state["hook"] = _ntff_profile_via_ctypes("/opt/axon/libaxon_pjrt.so")
    except Exception:
        pass
    mod.get_axon_ntff_profile_hook = lambda: state["hook"]
    mod.set_axon_ntff_profile_hook = lambda h: state.__setitem__("hook", h)
    sys.modules["antenv.axon_hooks"] = mod


def _kernel_np(k, v, k_cache, v_cache, k_scale, k_zero, v_scale, v_zero, start_pos):
    """Pure-numpy fallback for shapes the bass path doesn't handle."""
    def qp(x):
        mn = x.min(-1, keepdims=True)
        mx = x.max(-1, keepdims=True)
        scale = np.maximum((mx - mn) / np.float32(15.0), np.float32(1e-8))
        zero = -mn / scale
        q = np.clip(np.round(x / scale + zero), 0, 15).astype(np.uint8)
        return (q[..., 0::2] | (q[..., 1::2] << 4)), scale[..., 0], zero[..., 0]

    def dq(p, s, z):
        lo = (p & 15).astype(np.float32)
        hi = ((p >> 4) & 15).astype(np.float32)
        q = np.stack([lo, hi], -1).reshape(p.shape[:-1] + (p.shape[-1] * 2,))
        return (q - z[..., None]) * s[..., None]

    S = int(start_pos)
    E = S + k.shape[2]
    outs = []
    for x, cache, sc, zp in ((k, k_cache, k_scale, k_zero), (v, v_cache, v_scale, v_zero)):
        pp, ps, pz = qp(x)
        cache = cache.copy(); sc = sc.copy(); zp = zp.copy()
        cache[:, :, S:E] = pp
        sc[:, :, S:E] = ps
        zp[:, :, S:E] = pz
        outs.append(dq(cache[:, :, :E], sc[:, :, :E], zp[:, :, :E]))
    return tuple(outs)


def kernel(k, v, k_cache, v_cache, k_scale, k_zero, v_scale, v_zero, start_pos,
           _trace=False):
    k = np.asarray(k, np.float32)
    v = np.asarray(v, np.float32)
    k_cache = np.asarray(k_cache, np.uint8)
    v_cache = np.asarray(v_cache, np.uint8)
    k_scale = np.asarray(k_scale, np.float32)
    k_zero = np.asarray(k_zero, np.float32)
    v_scale = np.asarray(v_scale, np.float32)
    v_zero = np.asarray(v_zero, np.float32)
    S = int(start_pos)

    if (k.shape != (B, H, L, D) or S % 128 or S + L > MAX_SEQ):
        return _kernel_np(k, v, k_cache, v_cache, k_scale, k_zero, v_scale, v_zero, S)

    nc = _get_nc(S)
    E = S + L

    in_maps = []
    for m in range(N_CORES):
        hs = slice(m * HC, (m + 1) * HC)
        im = {
            "xk": np.ascontiguousarray(k[:, hs]),
            "xv": np.ascontiguousarray(v[:, hs]),
        }
        if S:
            im["pk"] = np.ascontiguousarray(k_cache[:, hs, :S, :])
            im["pv"] = np.ascontiguousarray(v_cache[:, hs, :S, :])
            im["sck"] = np.ascontiguousarray(k_scale[:, hs, :S])
            im["zpk"] = np.ascontiguousarray(k_zero[:, hs, :S])
            im["scv"] = np.ascontiguousarray(v_scale[:, hs, :S])
            im["zpv"] = np.ascontiguousarray(v_zero[:, hs, :S])
        in_maps.append(im)

    if _trace:
        _install_ntff_hook_shim()
    res = run_bass_kernel_spmd(nc, in_maps, list(range(N_CORES)), trace=_trace)

    k_dec = np.empty((B, H, E, D), np.float32)
    v_dec = np.empty((B, H, E, D), np.float32)
    for m in range(N_CORES):
        hs = slice(m * HC, (m + 1) * HC)
        k_dec[:, hs] = res.results[m]["ok"]
        v_dec[:, hs] = res.results[m]["ov"]
    if _trace:
        return (k_dec, v_dec), res
    return k_dec, v_dec


# revision 7
# speedup vs baseline: 1.3398x; 1.0826x over previous
"""CompressedKVCache kernel for Trainium2 (8 NeuronCores, head-sharded).

Computes, per (b, h) head:
  quantize k/v rows to int4 (per-row min/max affine), scatter into a
  uint8-packed cache at [start_pos : start_pos+L), then dequantize the
  cache prefix [0 : start_pos+L) back to f32.

Sharding: H=32 heads split across 8 cores (4 heads each); everything is
independent per head, no cross-core communication.

The packed cache itself is never returned, so the [start, end) region is
quantize->dequantized entirely on-chip; only the [0, start) prefix is read
from the cache inputs.

Layout: all DRAM<->SBUF transfers use a "(p c)" row blocking (partition p
owns 16 *consecutive* rows as column chunks) so every DMA descriptor is a
large contiguous run (1-8 KiB).  Engine balance: min/max reduces + most of
the quant-region dequant on DVE, quant round (f32->u8 RNE) + the rest of
the dequant on ACT, nibble unpack + prefix dequant on GpSimd.
"""

import sys

sys.path.insert(0, "/opt/trn_rl_repo")

import numpy as np
from concourse import bass, mybir
from concourse import tile
from concourse.bass_utils import run_bass_kernel_spmd

F32 = mybir.dt.float32
U8 = mybir.dt.uint8
U32 = mybir.dt.uint32
I32 = mybir.dt.int32
Alu = mybir.AluOpType
Act = mybir.ActivationFunctionType
AX = mybir.AxisListType
INV15 = float(np.float32(1.0 / 15.0))

B, H, L, D = 2, 32, 2048, 128
MAX_SEQ = 8192
N_CORES = 8
HC = H // N_CORES  # heads per core

# --- tuning knobs -----------------------------------------------------------
RG = 8           # chunks per min/max reduce op (divisor of 16)
DQR_ACT = 5      # per tensor: dequant chunks on ACT (q*scale + mn)
DQR_GPS = 8      # per tensor: dequant chunks on GpSimd (q*scale + mn)
PRE_DVE = 5      # per tensor: prefix-dequant chunks on DVE; rest GpSimd
Q_DTYPE = U8     # quantized value dtype (ACT output convert does the round)


def _split_multiwait(nc):
    """This container's walrus accepts only ONE sync-wait per instruction;
    Tile's tail drain (and occasionally other insts) carry several. Split
    extras into single-wait EventSemaphore insts inserted just before."""
    for fn in nc.m.functions:
        for blk in fn.blocks:
            out = []
            for ins in blk.instructions:
                si = ins.sync_info
                if si is not None and si.on_wait is not None and len(si.on_wait) > 1:
                    waits = list(si.on_wait)
                    for j, w in enumerate(waits[:-1]):
                        out.append(mybir.InstEventSemaphore(
                            name=f"{ins.name}_sw{j}", ins=[], outs=[],
                            engine=ins.engine,
                            sync_info=mybir.SyncInfo(on_wait=[w], on_update=[])))
                    si.on_wait = [waits[-1]]
                    ins.sync_info = si
                out.append(ins)
            blk.instructions = out


def _build(start_pos: int):
    """Trace the per-core Bass kernel for a given start_pos.

    Per core: xk/xv (B,HC,L,D) f32, prefix packed caches (B,HC,S,64) u8 and
    prefix scale/zero rows (B,HC,S) f32 -> ok/ov (B,HC,S+L,D) f32.
    """
    S = start_pos
    E = S + L
    CQ = L // 128            # quant row-chunks per head
    CP = S // 128            # prefix row-chunks per head
    assert L % 128 == 0 and S % 128 == 0 and E <= MAX_SEQ

    nc = bass.Bass(trn_type="TRN2")

    ins_q, ins_p, ins_sc, ins_zp, outs = {}, {}, {}, {}, {}
    for t in ("k", "v"):
        ins_q[t] = nc.dram_tensor(f"x{t}", [B, HC, L, D], F32, kind="ExternalInput")
        if S:
            ins_p[t] = nc.dram_tensor(f"p{t}", [B, HC, S, D // 2], U8, kind="ExternalInput")
            ins_sc[t] = nc.dram_tensor(f"sc{t}", [B, HC, S], F32, kind="ExternalInput")
            ins_zp[t] = nc.dram_tensor(f"zp{t}", [B, HC, S], F32, kind="ExternalInput")
        outs[t] = nc.dram_tensor(f"o{t}", [B, HC, E, D], F32, kind="ExternalOutput")

    pairs = [(b, hh) for b in range(B) for hh in range(HC)]
    P = len(pairs)
    st = [dict() for _ in range(P)]  # per-pair tile handles

    with tile.TileContext(nc) as tc:
        with tc.tile_pool(name="big", bufs=3) as big, \
             tc.tile_pool(name="small", bufs=3) as small:

            def loads(i):
                b, hh = pairs[i]
                s = st[i]
                if S:
                    s["sc"] = small.tile([128, 2 * CP], F32, tag="sc",
                                         name="sc")
                    s["zp"] = small.tile([128, 2 * CP], F32, tag="zp",
                                         name="zp")
                    for kv, t in enumerate(("k", "v")):
                        nc.sync.dma_start(
                            out=s["sc"][:, kv * CP:(kv + 1) * CP],
                            in_=ins_sc[t][b, hh, :].rearrange(
                                "(p c) -> p c", p=128))
                        nc.sync.dma_start(
                            out=s["zp"][:, kv * CP:(kv + 1) * CP],
                            in_=ins_zp[t][b, hh, :].rearrange(
                                "(p c) -> p c", p=128))
                    s["pk2"] = big.tile([128, 2, CP, D // 2], U8, tag="pk",
                                        name="pk2")
                    for kv, t in enumerate(("k", "v")):
                        nc.sync.dma_start(
                            out=s["pk2"][:, kv, :, :],
                            in_=ins_p[t][b, hh, :, :].rearrange(
                                "(p c) d -> p c d", p=128))
                # partition p <- rows p*16..p*16+15
                s["xkv"] = big.tile([128, 2 * CQ, D], F32, tag="xkv",
                                    name="xkv")
                for kv, t in enumerate(("k", "v")):
                    nc.sync.dma_start(
                        out=s["xkv"][:, kv * CQ:(kv + 1) * CQ, :],
                        in_=ins_q[t][b, hh, :, :].rearrange(
                            "(p c) d -> p c d", p=128))

            def front(i):
                """DVE front work for pair i: pnz, unpack, reduces, stats."""
                s = st[i]
                if S:
                    s["pnz"] = small.tile([128, 2 * CP], F32, tag="pnz",
                                          name="pnz")
                    nc.vector.tensor_tensor(out=s["pnz"][:, :],
                                            in0=s["zp"][:, :],
                                            in1=s["sc"][:, :], op=Alu.mult)
                    nc.vector.tensor_scalar(out=s["pnz"][:, :],
                                            in0=s["pnz"][:, :],
                                            scalar1=-1.0, scalar2=None,
                                            op0=Alu.mult)
                    s["lohi"] = big.tile([128, 2, CP, D], U8, tag="lohi",
                                         name="lohi")
                    for kv in range(2):
                        # u32-lane nibble unpack: lo -> cols 0:64,
                        # hi -> cols 64:128 per row
                        pk32 = s["pk2"][:, kv, :, :].bitcast(U32)
                        nc.vector.tensor_scalar(
                            out=s["lohi"][:, kv, :, 0:D // 2].bitcast(U32),
                            in0=pk32, scalar1=0x0F0F0F0F, scalar2=None,
                            op0=Alu.bitwise_and)
                        nc.vector.tensor_scalar(
                            out=s["lohi"][:, kv, :, D // 2:D].bitcast(U32),
                            in0=pk32, scalar1=4, scalar2=0x0F0F0F0F,
                            op0=Alu.logical_shift_right,
                            op1=Alu.bitwise_and)
                # batched min/max reduces
                xkv = s["xkv"]
                mn = small.tile([128, 2 * CQ], F32, tag="mn", name="mn")
                mx = small.tile([128, 2 * CQ], F32, tag="mx", name="mx")
                s["mn"], s["mx"] = mn, mx
                for g in range(0, 2 * CQ, RG):
                    nc.vector.tensor_reduce(
                        out=mx[:, g:g + RG], in_=xkv[:, g:g + RG, :],
                        axis=AX.X, op=Alu.max)
                    nc.vector.tensor_reduce(
                        out=mn[:, g:g + RG], in_=xkv[:, g:g + RG, :],
                        axis=AX.X, op=Alu.min)
                # stats chain
                scale = small.tile([128, 2 * CQ], F32, tag="scale",
                                   name="scale")
                nc.vector.tensor_tensor(out=scale[:, :], in0=mx[:, :],
                                        in1=mn[:, :], op=Alu.subtract)
                nc.vector.tensor_scalar(out=scale[:, :], in0=scale[:, :],
                                        scalar1=INV15, scalar2=1e-8,
                                        op0=Alu.mult, op1=Alu.max)
                rcp = small.tile([128, 2 * CQ], F32, tag="rcp", name="rcp")
                nc.vector.reciprocal(out=rcp[:, :], in_=scale[:, :])
                zero = small.tile([128, 2 * CQ], F32, tag="zero", name="zero")
                nc.vector.tensor_scalar(out=zero[:, :], in0=mn[:, :],
                                        scalar1=-1.0, scalar2=None,
                                        op0=Alu.mult)
                nc.vector.tensor_tensor(out=zero[:, :], in0=zero[:, :],
                                        in1=rcp[:, :], op=Alu.mult)
                s["scale"], s["rcp"], s["zero"] = scale, rcp, zero

            def back(i):
                """Quant + dequants + stores for pair i."""
                b, hh = pairs[i]
                s = st[i]
                xkv = s["xkv"]
                mn, scale, rcp, zero = s["mn"], s["scale"], s["rcp"], s["zero"]
                os_p, os_q = {}, {}
                for kv, t in enumerate(("k", "v")):
                    if S:
                        os_p[t] = big.tile([128, CP, D], F32, tag=f"op{kv}",
                                           name=f"op{kv}")
                    os_q[t] = big.tile([128, CQ, D], F32, tag=f"oq{kv}",
                                       name=f"oq{kv}")

                # quant: ACT Identity, u8 out (RNE round in convert)
                q8 = big.tile([128, 2 * CQ, D], Q_DTYPE, tag="q", name="q8")
                for kv, t in enumerate(("k", "v")):
                    for c in range(CQ):
                        cc = kv * CQ + c
                        nc.scalar.activation(
                            out=q8[:, cc, :], in_=xkv[:, cc, :],
                            func=Act.Identity,
                            bias=zero[:, cc:cc + 1],
                            scale=rcp[:, cc:cc + 1])

                # prefix dequant + interleave per chunk (strided dst)
                if S:
                    for kv, t in enumerate(("k", "v")):
                        o = os_p[t]
                        for c in range(CP):
                            cc = kv * CP + c
                            src = s["lohi"][:, kv, c, :].rearrange(
                                "p (two d) -> p two d", two=2)
                            dst = o[:, c, :].rearrange(
                                "p (d two) -> p two d", two=2)
                            eng = nc.vector if c < PRE_DVE else nc.gpsimd
                            eng.tensor_scalar(
                                out=dst, in0=src,
                                scalar1=s["sc"][:, cc:cc + 1],
                                scalar2=s["pnz"][:, cc:cc + 1],
                                op0=Alu.mult, op1=Alu.add)
                        # prefix store fires as soon as this half is done
                        o_dram = outs[t][b, hh, 0:S, :].rearrange(
                            "(p c) d -> p c d", p=128)
                        nc.sync.dma_start(out=o_dram, in_=o[:, :, :])

                # quant-region dequant, split ACT / GpSimd / DVE
                for kv, t in enumerate(("k", "v")):
                    o = os_q[t]
                    for c in range(CQ):
                        cc = kv * CQ + c
                        if c < DQR_ACT:
                            # q*scale + mn  (== (q - zero)*scale)
                            nc.scalar.activation(
                                out=o[:, c, :], in_=q8[:, cc, :],
                                func=Act.Identity,
                                bias=mn[:, cc:cc + 1],
                                scale=scale[:, cc:cc + 1])
                        elif c < DQR_ACT + DQR_GPS:
                            nc.gpsimd.tensor_scalar(
                                out=o[:, c, :], in0=q8[:, cc, :],
                                scalar1=scale[:, cc:cc + 1],
                                scalar2=mn[:, cc:cc + 1],
                                op0=Alu.mult, op1=Alu.add)
                        else:
                            nc.vector.tensor_scalar(
                                out=o[:, c, :], in0=q8[:, cc, :],
                                scalar1=zero[:, cc:cc + 1],
                                scalar2=scale[:, cc:cc + 1],
                                op0=Alu.subtract, op1=Alu.mult)
                    o_dram = outs[t][b, hh, S:E, :].rearrange(
                        "(p c) d -> p c d", p=128)
                    nc.sync.dma_start(out=o_dram, in_=o[:, :, :])
                st[i] = None  # release handles

            # software pipeline: loads 2 ahead, DVE front work 1 ahead
            for i in range(P + 2):
                if i < P:
                    loads(i)
                if 1 <= i <= P:
                    front(i - 1)
                if i >= 2:
                    back(i - 2)

    _split_multiwait(nc)
    return nc


_CACHE = {}


def _get_nc(start_pos: int):
    if start_pos not in _CACHE:
        _CACHE[start_pos] = _build(start_pos)
    return _CACHE[start_pos]


def _install_ntff_hook_shim():
    """The agent image's antenv lacks axon_hooks; recreate it so
    run_bass_kernel_spmd(trace=True) can drive NTFF profiling."""
    import types
    if "antenv.axon_hooks" in sys.modules:
        return
    mod = types.ModuleType("antenv.axon_hooks")
    state = {"hook": None}
    try:
        from trn_agent_boot.trn_boot import _ntff_profile_via_ctypes
        state["hook"] = _ntff_profile_via_ctypes("/opt/axon/libaxon_pjrt.so")
    except Exception:
        pass
    mod.get_axon_ntff_profile_hook = lambda: state["hook"]
    mod.set_axon_ntff_profile_hook = lambda h: state.__setitem__("hook", h)
    sys.modules["antenv.axon_hooks"] = mod


def _kernel_np(k, v, k_cache, v_cache, k_scale, k_zero, v_scale, v_zero, start_pos):
    """Pure-numpy fallback for shapes the bass path doesn't handle."""
    def qp(x):
        mn = x.min(-1, keepdims=True)
        mx = x.max(-1, keepdims=True)
        scale = np.maximum((mx - mn) / np.float32(15.0), np.float32(1e-8))
        zero = -mn / scale
        q = np.clip(np.round(x / scale + zero), 0, 15).astype(np.uint8)
        return (q[..., 0::2] | (q[..., 1::2] << 4)), scale[..., 0], zero[..., 0]

    def dq(p, s, z):
        lo = (p & 15).astype(np.float32)
        hi = ((p >> 4) & 15).astype(np.float32)
        q = np.stack([lo, hi], -1).reshape(p.shape[:-1] + (p.shape[-1] * 2,))
        return (q - z[..., None]) * s[..., None]

    S = int(start_pos)
    E = S + k.shape[2]
    outs = []
    for x, cache, sc, zp in ((k, k_cache, k_scale, k_zero), (v, v_cache, v_scale, v_zero)):
        pp, ps, pz = qp(x)
        cache = cache.copy(); sc = sc.copy(); zp = zp.copy()
        cache[:, :, S:E] = pp
        sc[:, :, S:E] = ps
        zp[:, :, S:E] = pz
        outs.append(dq(cache[:, :, :E], sc[:, :, :E], zp[:, :, :E]))
    return tuple(outs)


def kernel(k, v, k_cache, v_cache, k_scale, k_zero, v_scale, v_zero, start_pos,
           _trace=False):
    k = np.asarray(k, np.float32)
    v = np.asarray(v, np.float32)
    k_cache = np.asarray(k_cache, np.uint8)
    v_cache = np.asarray(v_cache, np.uint8)
    k_scale = np.asarray(k_scale, np.float32)
    k_zero = np.asarray(k_zero, np.float32)
    v_scale = np.asarray(v_scale, np.float32)
    v_zero = np.asarray(v_zero, np.float32)
    S = int(start_pos)

    if (k.shape != (B, H, L, D) or S % 128 or S + L > MAX_SEQ):
        return _kernel_np(k, v, k_cache, v_cache, k_scale, k_zero, v_scale, v_zero, S)

    nc = _get_nc(S)
    E = S + L

    in_maps = []
    for m in range(N_CORES):
        hs = slice(m * HC, (m + 1) * HC)
        im = {
            "xk": np.ascontiguousarray(k[:, hs]),
            "xv": np.ascontiguousarray(v[:, hs]),
        }
        if S:
            im["pk"] = np.ascontiguousarray(k_cache[:, hs, :S, :])
            im["pv"] = np.ascontiguousarray(v_cache[:, hs, :S, :])
            im["sck"] = np.ascontiguousarray(k_scale[:, hs, :S])
            im["zpk"] = np.ascontiguousarray(k_zero[:, hs, :S])
            im["scv"] = np.ascontiguousarray(v_scale[:, hs, :S])
            im["zpv"] = np.ascontiguousarray(v_zero[:, hs, :S])
        in_maps.append(im)

    if _trace:
        _install_ntff_hook_shim()
    res = run_bass_kernel_spmd(nc, in_maps, list(range(N_CORES)), trace=_trace)

    k_dec = np.empty((B, H, E, D), np.float32)
    v_dec = np.empty((B, H, E, D), np.float32)
    for m in range(N_CORES):
        hs = slice(m * HC, (m + 1) * HC)
        k_dec[:, hs] = res.results[m]["ok"]
        v_dec[:, hs] = res.results[m]["ov"]
    if _trace:
        return (k_dec, v_dec), res
    return k_dec, v_dec
